# revision 36
# baseline (speedup 1.0000x reference)
"""Bahdanau attention forward on 8 Trainium2 NeuronCores (data-parallel).

Layout: value rows flattened to r = b*W + w, placed at partition p = r % 120
(cols = r // 120), padded to 128 partitions (8 dead rows, masked out).  Each
column j holds the 120 rows of a 6-batch window (b = 6j + p//20, w = p%20).

Per-core pipeline, G=64 columns per block:
  1. fp16 loads: value tile VT and a host-replicated query tensor QT
     (qin[r] = (q*W1)[b(r)] replicated over w, optionally pre-folded with
     W2 -- pure replication / small-weight fold done on host)
  2. h = VT*QT (one DVE fp16 2x tensor_tensor; w2rep variant adds one more)
  3. t = tanh(h) in place on ScalarE
  4. scores via custom DVE op ANT_MUL_SSCAN: global inclusive cumsum of
     t*w3rep along the stream; per-row sums are differences of consecutive
     row-end elements (one e-segment per column)
  5. softmax over w WITHOUT leaving the layout, using tiny PE matmuls:
     denom[b_sub, col] via a constant [128,6] summing stationary; 1/denom
     on DVE; replicated back to 120 partitions via a constant [6,128]
     stationary; a = e * recrep (small DVE ops)
  6. ablk = a * mask6 gives the block-diagonal moving operand [128, 6] per
     column; context^T accumulates in PSUM via PE matmuls with the VALUE
     TILE ITSELF as the stationary operand (FWL fp16 128x128 weight loads):
     out[e, b_sub] = sum_p VT[p, e] * ablk[p, b_sub]
  7. PSUM -> SBUF drains on ScalarE (fp32->fp16), one fp16 store of ctx^T;
     host transposes back.

Engine budget per core (est): DVE ~300us, ACT ~170us, PE ~115us, DMA ~250us.
"""

import numpy as np

B, W, E = 65536, 20, 128
N_CORES = 8
B_CORE = B // N_CORES

R_CORE = B_CORE * W          # 163840 value rows per core
P = 120                      # live partitions (dead: 120..127)
RPP = 2                      # value rows per partition (DMA run = RPP*256B)
BW = 6 * RPP                 # batches per column window
RPC = P * RPP                # value rows per column
NCOL = (R_CORE + RPC - 1) // RPC
NFULL = R_CORE // RPC        # full columns
TAIL_ROWS = R_CORE - NFULL * RPC     # rows in the partial last column
TAIL_PARTS = TAIL_ROWS // RPP        # live partitions in the last column
G = 24                       # columns per block (G*BW=288 fp32 psum cols)
VBUFS = 5                    # rotating VQ buffers (pipeline depth)
NBLK = (NCOL + G - 1) // G
CTX_COLS = NCOL * BW         # >= B_CORE

# If True, host folds W2 into the replicated query (saves one DVE pass).
QFOLD = True
ABLK_GPSIMD = True
DRAIN_DVE = False
H_GPSIMD = True   # h-mult on Pool (DVE is the bottleneck engine)

# B-class blocks DMA [v0 | qin0 | v1] rows (768B per (p,col) descriptor, 3/4
# the bytes of A-class [v0 | qin0 | v1 | qin1]) and rebuild the k=1 qin slot
# on-device as qin0 * ratio, where ratio[p,e] = W2[w(p,1),e]/W2[w(p,0),e] is
# a per-partition fp32 constant -- one half-size tensor_tensor on Pool.  The
# tail block is always A-class.
NB_B = 24

def _class_b(kb):
    if kb >= NBLK - 1:
        return False
    return ((kb + 1) * NB_B) // (NBLK - 1) > (kb * NB_B) // (NBLK - 1)

CLASSB = [_class_b(kb) for kb in range(NBLK)]
CB0 = []
_c = 0
for _kb in range(NBLK):
    CB0.append(_c)
    if CLASSB[_kb]:
        _c += G
NCOLB = _c
COLS_B = [c for kb in range(NBLK) if CLASSB[kb] for c in range(kb * G, kb * G + G)]

_CACHE = {}


def _register_scan_op():
    """Custom DVE op: global inclusive cumsum of in0*in1 (fp32 feedback).

    Does NOT reset at subdim row boundaries; callers recover per-row segment
    sums as differences of consecutive row-end elements.
    """
    import re

    import concourse.dve_ops as dops
    from concourse import dve_spec as ds

    for o in dops.OPS:
        if o.name == "ANT_MUL_SSCAN":
            return o

    def _ref(in0, in1, c0, c1, c2):
        x = in0.astype(np.float32) * in1.astype(np.float32)
        return np.cumsum(x, axis=-1)

    spec = ds.Spec(
        body=ds.Scan(ds.AluOp.ADD, ds.Src0 * ds.Src1), reference=_ref
    )
    op = dops.DveOp("ANT_MUL_SSCAN", spec, subdim=True, uops_sha={})
    dops.OPS.append(op)
    dops._SUB_OPCODE_FOR_NAME[op.name] = dops._CUSTOM_DVE_ROW_BASE + len(dops.OPS) - 1
    for ver in ("v3", "v4"):
        try:
            op.compile(ver)
        except ValueError as e:
            m = re.search(r'"([0-9a-f]{16})"', str(e))
            if not m:
                raise
            op.uops_sha[ver] = m.group(1)
            op.compile(ver)
    return op


def _build(
    b_core: int,
    reps: int = 1,
    skip_mm: bool = False,
    skip_scan: bool = False,
    skip_big_dma: bool = False,
    skip_m1: bool = False,
    n_dev: int = N_CORES,
):
    import sys

    if "/opt/trn_rl_repo" not in sys.path:
        sys.path.insert(0, "/opt/trn_rl_repo")
    import concourse.bacc as bacc
    import concourse.bass as bass
    import concourse.mybir as mybir
    import concourse.tile as tile

    assert b_core == B_CORE

    f16 = mybir.dt.float16
    f32 = mybir.dt.float32

    sscan = _register_scan_op()

    nc = bacc.Bacc(
        "TRN2",
        target_bir_lowering=False,
        debug=False,
        enable_asserts=False,
        num_devices=n_dev,
    )

    E2 = RPP * E
    vq_d = nc.dram_tensor("vq", [R_CORE, 2 * E], f16, kind="ExternalInput").ap()
    vqb_d = nc.dram_tensor(
        "vqb", [max(NCOLB, 1) * P, 3 * E], f16, kind="ExternalInput"
    ).ap()
    w3rep_d = nc.dram_tensor("w3rep", [128, E2], f16, kind="ExternalInput").ap()
    msum_d = nc.dram_tensor("msum", [128, BW], f32, kind="ExternalInput").ap()
    repm_d = nc.dram_tensor("repm", [BW, 128], f32, kind="ExternalInput").ap()
    maskb_d = nc.dram_tensor("maskb", [128, BW], f16, kind="ExternalInput").ap()
    w2rep_d = nc.dram_tensor("w2rep", [128, E2], f16, kind="ExternalInput").ap()
    ratio_d = nc.dram_tensor("ratio", [128, E], f32, kind="ExternalInput").ap()
    zeros_d = nc.dram_tensor(
        "zeros", [8, G * RPP * 2 * E], f16, kind="ExternalInput"
    ).ap()
    ctxT_d = nc.dram_tensor("ctxT", [E, B_CORE], f16, kind="ExternalOutput").ap()

    mult = mybir.AluOpType.mult
    sub = mybir.AluOpType.subtract
    Tanh = mybir.ActivationFunctionType.Tanh
    Exp = mybir.ActivationFunctionType.Exp

    with tile.TileContext(nc) as tc:
        with (
            tc.tile_pool(name="consts", bufs=1) as cpool,
            tc.tile_pool(name="vbuf", bufs=VBUFS) as vpool,
            tc.tile_pool(name="csbuf", bufs=1) as cspool,
            tc.tile_pool(name="small", bufs=2) as spool,
            tc.tile_pool(name="ctxps", bufs=2, space="PSUM") as cps,
            tc.tile_pool(name="smps", bufs=2, space="PSUM") as sps,
        ):
            w3t = cpool.tile([128, E2], f16, tag="w3t")
            nc.sync.dma_start(w3t[:], w3rep_d)
            msum = cpool.tile([128, BW], f32, tag="msum")
            nc.sync.dma_start(msum[:], msum_d)
            repm = cpool.tile([BW, 128], f32, tag="repm")
            nc.sync.dma_start(repm[:], repm_d)
            maskb = cpool.tile([128, BW], f16, tag="maskb")
            nc.sync.dma_start(maskb[:], maskb_d)
            w2t = cpool.tile([128, E2], f16, tag="w2t")
            nc.sync.dma_start(w2t[:], w2rep_d)
            ratio_t = cpool.tile([128, E], f32, tag="ratio")
            nc.sync.dma_start(ratio_t[:], ratio_d)
            ctxT = cpool.tile([128, CTX_COLS], f16, tag="ctxT")

            w3b = w3t[:].unsqueeze(1).broadcast_to([128, G, E2])
            w3b4 = (
                w3t[:]
                .rearrange("p (k e) -> p k e", k=RPP)
                .unsqueeze(1)
                .broadcast_to([128, G, RPP, E])
            )
            w2b = w2t[:].unsqueeze(1).broadcast_to([128, G, E2])
            mbb = (
                maskb[:]
                .unsqueeze(1)
                .unsqueeze(2)
                .broadcast_to([128, G, RPP, BW])
            )

            AXX = mybir.AxisListType.X
            add = mybir.AluOpType.add

            for it in range(NBLK * reps):
                kb = it % NBLK
                col0 = kb * G
                g = min(G, NCOL - col0)
                grpp = g * RPP
                gf = g if kb < NBLK - 1 else g - 1  # full columns
                r0 = col0 * RPC

                # interleaved tile: VQ[p, j, k, 0, :] = value row, [.., 1, :]
                # the matching (replicated) query row
                VQ = vpool.tile([128, G, RPP, 2, E], f16)
                VTv = VQ[:, :, :, 0, :]
                QTv = VQ[:, :, :, 1, :]
                E32 = spool.tile([128, G * RPP], f32, tag="e32")
                if it < VBUFS:
                    # zero the dead partitions (120..127) of each rotating
                    # buffer once via a tiny DMA (an engine memset costs
                    # free-size cycles regardless of partition count): NaN
                    # garbage there would poison the denominator/context
                    # matmuls via 0*NaN.
                    nc.sync.dma_start(
                        VQ[120:, :, :, :, :].rearrange(
                            "p g k c e -> p g (k c e)"
                        ),
                        zeros_d[:].rearrange(
                            "p (g r) -> p g r", r=RPP * 2 * E
                        ),
                    )
                if it < 2:
                    nc.vector.memset(E32[96:, :], 0.0)
                if kb == NBLK - 1:
                    nc.vector.memset(VQ[:, gf, :, :, :], 0.0)
                isB = CLASSB[kb]
                VQf = VQ[:].rearrange("p g k c e -> p g (k c) e")
                if not skip_big_dma:
                    if isB:
                        rb0 = CB0[kb] * P
                        nc.sync.dma_start(
                            VQf[:P, 0:gf, 0:3, :],
                            vqb_d[rb0 : rb0 + P * gf, :].rearrange(
                                "(g p) (s e) -> p g s e", p=P, s=3
                            ),
                        )
                    else:
                        nc.sync.dma_start(
                            VQ[:P, 0:gf, :, :, :],
                            vq_d[r0 : r0 + RPC * gf, :].rearrange(
                                "(g p k) (c e) -> p g k c e", p=P, k=RPP, c=2
                            ),
                        )
                if kb == NBLK - 1:
                    rt = r0 + RPC * gf
                    nc.sync.dma_start(
                        VQ[:TAIL_PARTS, gf, :, :, :],
                        vq_d[rt : rt + TAIL_ROWS, :].rearrange(
                            "(p k) (c e) -> p k c e", p=TAIL_PARTS, c=2
                        ),
                    )

                # h = v * qin  (in place over the query slot)
                h_eng = nc.gpsimd if H_GPSIMD else nc.vector
                if not skip_m1:
                    if isB:
                        # rebuild the k=1 qin slot: slot3 <- qin0 * ratio
                        rb = (
                            ratio_t[:]
                            .unsqueeze(1)
                            .broadcast_to([128, G, E])
                        )
                        nc.gpsimd.tensor_tensor(
                            VQf[:, 0:g, 3, :], VQf[:, 0:g, 1, :],
                            rb[:, 0:g], mult,
                        )
                    h_eng.tensor_tensor(
                        QTv[:, 0:g], QTv[:, 0:g], VTv[:, 0:g], mult
                    )
                    nc.scalar.activation(QTv[:, 0:g], QTv[:, 0:g], Tanh)

                # scores: per-k-slot cumsum(t*w3) along the (col, e) stream;
                # per-row sums are diffs of consecutive row-end elements
                # within each slot's stream
                SC = spool.tile([128, G * RPP], f32, tag="sc")
                if skip_scan and it < 2:
                    nc.vector.memset(SC[:], 0.0)
                if not skip_scan:
                    CS = cspool.tile([128, G, E2], f32)
                    csv = CS[:].rearrange("p g (k e) -> p g k e", k=RPP)
                    scv = SC[:].rearrange("p (g k) -> p g k", k=RPP)
                    for k in range(RPP):
                        w3k = (
                            w3t[:, k * E : (k + 1) * E]
                            .unsqueeze(1)
                            .broadcast_to([128, G, E])
                        )
                        nc.vector._custom_dve(
                            sscan, out=csv[:, 0:g, k], in0=QTv[:, 0:g, k],
                            in1=w3k[:, 0:g]
                        )
                    # row-end diffs, both k slots in one copy + one sub
                    cend = csv[:, 0:g, :, E - 1]  # [128, g, RPP]
                    nc.vector.tensor_copy(scv[:, 0:1, :], cend[:, 0:1, :])
                    nc.vector.tensor_tensor(
                        scv[:, 1:g, :], cend[:, 1:g, :], cend[:, 0 : g - 1, :],
                        sub,
                    )

                nc.scalar.activation(E32[:P, 0:grpp], SC[:P, 0:grpp], Exp)

                # softmax over w via PE: denom -> 1/denom -> replicate
                E32r = E32[:].rearrange("p (g k) -> p g k", k=RPP)
                ES = spool.tile([128, G], f32, tag="esum")
                nc.vector.tensor_reduce(ES[:, 0:g], E32r[:, 0:g, :], AXX, add)
                DM = sps.tile([BW, G], f32)
                nc.tensor.matmul(DM[:, 0:g], msum[:], ES[:, 0:g])
                REC = spool.tile([BW, G], f32, tag="rec")
                nc.vector.reciprocal(REC[:, 0:g], DM[:, 0:g])
                RR = sps.tile([128, G], f32)
                nc.tensor.matmul(RR[:, 0:g], repm[:], REC[:, 0:g])
                A4 = spool.tile([128, G, RPP], f16, tag="a4")
                rrb = RR[:].unsqueeze(2).broadcast_to([128, G, RPP])
                nc.vector.tensor_tensor(
                    A4[:, 0:g, :], E32r[:, 0:g, :], rrb[:, 0:g, :], mult
                )

                # block-diagonal moving operand, then context matmuls
                ABLK = spool.tile([128, G, RPP, BW], f16, tag="ablk")
                ab = A4[:].unsqueeze(3).broadcast_to([128, G, RPP, BW])
                ablk_eng = nc.gpsimd if ABLK_GPSIMD else nc.vector
                ablk_eng.tensor_tensor(
                    ABLK[:, 0:g, :, :], ab[:, 0:g, :, :], mbb[:, 0:g, :, :], mult
                )

                if not skip_mm:
                    CTXP = cps.tile([128, G * BW], f32)
                    for j in range(g):
                        for k in range(RPP):
                            nc.tensor.matmul(
                                CTXP[:, BW * j : BW * (j + 1)],
                                VTv[:, j, k, :],
                                ABLK[:, j, k, :],
                                start=(k == 0),
                                stop=(k == RPP - 1),
                            )

                    if DRAIN_DVE:
                        nc.vector.tensor_copy(
                            ctxT[:, BW * col0 : BW * (col0 + g)],
                            CTXP[:, 0 : BW * g],
                        )
                    else:
                        nc.scalar.copy(
                            ctxT[:, BW * col0 : BW * (col0 + g)],
                            CTXP[:, 0 : BW * g],
                        )

                # stream finished context columns out as they complete so the
                # output DMA overlaps the pipeline instead of tailing it
                if reps == 1 and (kb % 6 == 5 or kb == NBLK - 1):
                    c_lo = (kb - kb % 6) * G * BW
                    c_hi = min((kb + 1) * G * BW, B_CORE)
                    if c_hi > c_lo:
                        nc.sync.dma_start(
                            ctxT_d[:, c_lo:c_hi], ctxT[:, c_lo:c_hi]
                        )

            if skip_mm:
                nc.vector.memset(ctxT[:], 0.0)
            if reps != 1:
                nc.sync.dma_start(ctxT_d, ctxT[:, 0:B_CORE])

    nc.compile()
    return nc


def _get_nc(b_core: int):
    if b_core not in _CACHE:
        _CACHE[b_core] = _build(b_core)
    return _CACHE[b_core]


def _host_prep(query, value, W1, W2, W3):
    """Host-side prep: fp16 casts, query*W1 (tiny) replicated over w, and
    the small constant tensors."""
    q32 = np.asarray(query, dtype=np.float32)
    v32 = np.asarray(value, dtype=np.float32)
    W1 = np.asarray(W1, dtype=np.float32)
    W2 = np.asarray(W2, dtype=np.float32)
    W3 = np.asarray(W3, dtype=np.float32)

    vflat = np.ascontiguousarray(
        v32.reshape(B * W, E), dtype=np.float32
    ).astype(np.float16)

    rq = q32 * W1[0]  # [B, E]
    qin = (rq[:, None, :] * W2[None, :, :]).astype(np.float16)  # [B, W, E]
    qin = np.ascontiguousarray(qin.reshape(B * W, E))
    # interleave per row: vq[r] = [value_r | qin_r] so one DMA stream feeds
    # both operands (1KB descriptors, single issuing queue)
    vq = np.concatenate([vflat, qin], axis=1)  # [B*W, 2E] f16
    rq16 = rq.astype(np.float16)

    p = np.arange(128)
    live = p < P
    # partition p holds rows RPP*p .. RPP*p+RPP-1 (mod RPC) -> w indices
    w_of = (RPP * p[:, None] + np.arange(RPP)[None, :]) % W  # [128, RPP]
    w3rep = np.where(
        live[:, None, None], W3[w_of], 0.0
    ).reshape(128, RPP * E).astype(np.float16)
    w2rep = np.where(
        live[:, None, None], W2[w_of], 0.0
    ).reshape(128, RPP * E).astype(np.float16)
    bsub = (RPP * p) // W  # same for all RPP rows of a partition
    msum = (
        (bsub[:, None] == np.arange(BW)[None, :]) & live[:, None]
    ).astype(np.float32)
    maskb = msum.astype(np.float16)
    m = np.arange(128)
    repm = (
        ((RPP * m[None, :]) // W == np.arange(BW)[:, None]) & (m[None, :] < P)
    ).astype(np.float32)

    # ratio[p, e] = W2[w(p,1), e] / W2[w(p,0), e] in fp32: rebuilds the k=1
    # qin slot from the k=0 slot on-device
    w_of128 = (RPP * p[:, None] + np.arange(RPP)[None, :]) % W
    ratio = np.where(
        live[:, None], W2[w_of128[:, 1]] / W2[w_of128[:, 0]], 1.0
    ).astype(np.float32)

    return vq, vflat, w3rep, w2rep, ratio, msum, repm, maskb


def _make_vqb(vq_core):
    """B-class rows: [v(k=0) | qin(k=0) | v(k=1)] per (column, partition).

    vq_core rows are [v_r | qin_r]; row r=(col*240+2p) gives v0,qin0 and
    r+1 gives v1."""
    if NCOLB == 0:
        return np.zeros((P, 3 * E), np.float16)
    colg = np.asarray(COLS_B)[:, None]          # [NCOLB, 1]
    pp = np.arange(P)[None, :]
    r_k0 = (colg * RPC + 2 * pp).ravel()        # [NCOLB*P]
    return np.ascontiguousarray(
        np.concatenate(
            [vq_core[r_k0], vq_core[r_k0 + 1, :E]], axis=1
        )
    )


def make_in_maps(inputs):
    vq, vflat, w3rep, w2rep, ratio, msum, repm, maskb = _host_prep(
        inputs["query"], inputs["value"], inputs["W1"], inputs["W2"], inputs["W3"]
    )
    in_maps = []
    for c in range(N_CORES):
        rows = slice(c * R_CORE, (c + 1) * R_CORE)
        in_maps.append(
            {
                "vq": np.ascontiguousarray(vq[rows]),
                "vqb": _make_vqb(vq[rows]),
                "w3rep": w3rep,
                "msum": msum,
                "repm": repm,
                "maskb": maskb,
                "w2rep": w2rep,
                "ratio": ratio,
                "zeros": np.zeros((8, G * RPP * 2 * E), np.float16),
            }
        )
    return in_maps


def kernel(query, value, W1, W2, W3):
    import sys

    if "/opt/trn_rl_repo" not in sys.path:
        sys.path.insert(0, "/opt/trn_rl_repo")
    from concourse.bass_utils import run_bass_kernel_spmd

    inputs = {"query": query, "value": value, "W1": W1, "W2": W2, "W3": W3}
    in_maps = make_in_maps(inputs)
    nc = _get_nc(B_CORE)
    res = run_bass_kernel_spmd(nc, in_maps, list(range(N_CORES)))
    out = np.concatenate(
        [res.results[c]["ctxT"].T for c in range(N_CORES)], axis=0
    )
    return out.astype(np.float32)

